# revision 40
# baseline (speedup 1.0000x reference)
"""Trainium2 Bass kernel for nn_BlockCrossAttn (block-diagonal attention, E=H=1).

Math per (block b, batch n) pair (256-long vectors q', k', v_eff of the block):
    q' = wq*Q + bq ; k' = wk*K + bk ; v_eff = wo*(wv*V + bv) + bo
    soft[q,k] = softmax_k(q'[q] * k'[k])
    out[q] = sum_k soft[q,k] * v_eff[k]
(The V/out affine folds entirely into v_eff because softmax weights sum
to 1.)  No max-subtraction: |scores| <= ~27 worst case, exp is safe in fp32.

Sharding: 128 blocks of 256 rows; 16 blocks per core across 8 cores
(fully independent, no collectives).

All numeric prep happens HOST-side in make_in_maps (affine projections,
bf16 hi/lo splits, staging layout); the device module is weight- and
data-independent and is compiled exactly once.

Per-core device pipeline (512 pairs, act-groups of 3 pairs):
  - Scores via ONE bf16 matmul per (pair, k-half): contraction dim 3 with
    lhsT = [khi; klo; khi], rhs = [qhi; qhi; qlo], so
    S = khi*qhi + klo*qhi + khi*qlo = k*q exact to ~2^-18.  Streams at
    1 cycle/column (vs 2-4 passes for fp32/f32r).  The 3 pairs of a group
    are packed into PE row groups 0/32/64 (tile_position=(32s, 0)) and
    co-stream as waves of 3 concurrent matmuls into 3 distinct PSUM banks
    (t-outer order: a pair's two k-halves share a row group/bank and
    serialize; distinct pairs never share a bank while in flight).
  - ScalarE exp over [128, 1536] PSUM spans -> E (bf16) in SBUF.  ScalarE
    is the bottleneck engine (~90%+ busy); everything else hides under it.
    A tiny warm-up ACTIVATE at kernel start overlaps the ~2.7us exp
    table-load with the first DMAs.
  - PE reduction matmuls: lhsT = [ones, vhi, vlo] (v_eff split), rhs = E;
    the two k-halves accumulate in PSUM (start=(t==0), stop=(t==1)) ->
    [3, 256] rows (den, num_hi, num_lo) per pair; col-group packed 4
    pairs per result bank via tile_position=(0, 32j).
  - VectorE flushes result banks to SBUF rs; a DRAM scratch bounce
    re-lays 32 pairs into a dense [32, 768] tile; VectorE adds num
    partials, reciprocal_approx_fast + multiply; one contiguous DMA per
    block to the n-major output.  Division DMAs ride the gpsimd (SWDGE)
    queue so they never stall the sync-queue stage loads (which would
    inflate ACTIVATE durations via SBUF write contention); the last two
    blocks route through sync instead to avoid the SWDGE ring-drain cost
    at kernel exit.
"""

from contextlib import ExitStack

import numpy as np
import ml_dtypes

import concourse.bacc as bacc
import concourse.bass as bass
import concourse.tile as tile
from concourse import mybir
from concourse.bass_utils import run_bass_kernel_spmd

FP = mybir.dt.float32
BF16 = mybir.dt.bfloat16
AF = mybir.ActivationFunctionType
ALU = mybir.AluOpType

L = 32768          # sequence length
N = 32             # batch
BS = 256           # block size
NB = L // BS       # 128 blocks
NCORES = 8
BPC = NB // NCORES  # 16 blocks per core
LS = BPC * BS       # 4096 rows per core shard

GROUP = 3           # pairs per exp staging group (3 PSUM banks)
PAIRS = BPC * N     # 512 pairs per core
NGRP = (PAIRS + GROUP - 1) // GROUP  # 171 act groups (last has 2 pairs)
SPG = 4             # groups (= slots) per q/k stage DMA
PW = 512            # bf16 cols per pair in the qk stage (256 rhs + 2*128 lhsT)

BF_NP = ml_dtypes.bfloat16


def build_kernel_module(reps: int = 1) -> bass.Bass:
    """reps > 1 wraps the body in a device-side For_i loop (benchmarking)."""
    nc = bacc.Bacc("TRN2", target_bir_lowering=False, debug=False, num_devices=NCORES)
    # Score matmuls are packed 3-at-a-time into PE row groups 0/32/64
    # (tile_position=(32s, 0)); pair 3G+s uses row group s, i.e. SBUF
    # partitions 32s..32s+2.  qkst row r = 3s + c maps to partition 32s + c;
    # col slot = act-group index G.  Per pair slot (cols G*PW ..):
    #   cols 0:256   rows (qhi, qhi, qlo)    -> rhs [3, 256]
    #   cols 256:384 rows (khi0, klo0, khi0) -> lhsT t=0 [3, 128]
    #   cols 384:512 rows (khi1, klo1, khi1) -> lhsT t=1 [3, 128]
    qkst = nc.declare_dram_parameter("qkst", [9, NGRP * PW], BF16, isOutput=False)
    # vtab[p, b*192 + t*96 + n*3 + c]: c = (1.0, vhi, vlo) of v_eff[b*BS+t*128+p, n]
    vtab = nc.declare_dram_parameter("vtab", [128, BPC * 2 * N * 3], BF16, isOutput=False)
    out_t = nc.declare_dram_parameter("out_t", [N, LS], FP, isOutput=True)

    with tile.TileContext(nc) as tc:
        with ExitStack() as ctx:
            if reps == 1:
                _emit(ctx, tc, qkst, vtab, out_t)
            else:
                with tc.For_i(0, reps, 1):
                    _emit(ctx, tc, qkst, vtab, out_t)
    nc.compile()
    return nc


def _emit(ctx, tc, qkst, vtab, out_t):
    nc = tc.nc

    stage = ctx.enter_context(tc.tile_pool(name="stage", bufs=3))
    vpool = ctx.enter_context(tc.tile_pool(name="vpool", bufs=2))
    epool = ctx.enter_context(tc.tile_pool(name="epool", bufs=4))
    dpool = ctx.enter_context(tc.tile_pool(name="dpool", bufs=2))
    rspool = ctx.enter_context(tc.tile_pool(name="rspool", bufs=3))
    ps_stage = ctx.enter_context(tc.tile_pool(name="ps_stage", bufs=2, space="PSUM"))
    ps_res = ctx.enter_context(tc.tile_pool(name="ps_res", bufs=2, space="PSUM"))
    drs = ctx.enter_context(tc.tile_pool(name="drs", bufs=2, space="DRAM"))

    warm = dpool.tile([1, 8], FP, name="warm", tag="warm")
    nc.scalar.activation(warm[:], warm[:], AF.Exp)

    def load_stage(w):
        # SPG slots (= act groups); 9 dram rows -> partitions {32s + c}
        qs = stage.tile([128, SPG * PW], BF16, name="qs", tag="qs")
        w0 = w * SPG * PW
        width = min(SPG * PW, NGRP * PW - w0)
        # NB: one dma per row group — a single strided-partition view write
        # is not reliably ordered against the sliced matmul reads.
        for s in range(3):
            nc.sync.dma_start(
                out=qs[32 * s:32 * s + 3, 0:width],
                in_=qkst[3 * s:3 * s + 3, w0:w0 + width],
            )
        return qs

    def load_vtile(b):
        vt = vpool.tile([128, 2, N, 3], BF16, name="vt", tag="vt")
        nc.sync.dma_start(out=vt[:], in_=vtab[:, b * (2 * N * 3):(b + 1) * (2 * N * 3)])
        return vt

    # --- main loop --------------------------------------------------------------
    vcur = [None]
    res_state = {"tile": None, "count": 0, "nflush": 0, "rs": None, "first_g": 0}

    def emit_reduces(pend):
        e, members = pend
        for (s, b, n, vc) in members:
            g = b * N + n
            r = res_state["count"]
            if r == 0:
                res_state["tile"] = ps_res.tile([128, 256], FP, name="res", tag="res")
                if res_state["nflush"] == 0:
                    res_state["rs"] = rspool.tile([128, 2048], FP, name="rs", tag="rs")
                    res_state["first_g"] = g
            jj = r
            # the two k-halves accumulate in PSUM: rows = (den, num_hi, num_lo)
            for t in (0, 1):
                nc.tensor.matmul(
                    res_state["tile"][32 * jj:32 * jj + 3, 0:256],
                    lhsT=vc[:][:, t, n, :],
                    rhs=e[:][:, s * 512 + t * 256: s * 512 + (t + 1) * 256],
                    start=(t == 0), stop=(t == 1),
                    tile_position=(0, 32 * jj),
                )
            res_state["count"] += 1
            if res_state["count"] == 4:
                m = res_state["nflush"]
                nc.vector.tensor_copy(
                    res_state["rs"][:, m * 256:(m + 1) * 256], res_state["tile"][:]
                )
                res_state["count"] = 0
                res_state["tile"] = None
                res_state["nflush"] += 1
                last_block = res_state["first_g"] // N == BPC - 1
                if last_block and res_state["nflush"] == 4:
                    division_batch(half=0)
                elif res_state["nflush"] == 8:
                    division_batch(half=1 if last_block else None)

    def division_batch(half=None):
        b0 = res_state["first_g"] // N
        rs = res_state["rs"]
        # Last blocks route through queues that are idle by then: half 0 of
        # the final block via the scalar queue (free after the last ACT, and
        # emitted after it), half 1 + the previous block via sync; everything
        # else via gpsimd (SWDGE) to keep the sync queue clear of bursts.
        if half == 0:
            eng = nc.scalar
        elif half == 1 or b0 >= BPC - 2:
            eng = nc.sync
        else:
            eng = nc.gpsimd
        nm = 8 if half is None else 4
        m0 = 0 if half in (None, 0) else 4
        NP = 4 * nm  # batch rows (n) covered by these flushes
        scr = drs.tile([NP, 768], FP, name="scr", tag="scr")
        rsv = rs[:, m0 * 256:(m0 + nm) * 256].rearrange(
            "(j p2) (m q) -> j p2 m q", j=4, m=nm
        )
        sw = scr[:].rearrange("(m j) (r q) -> j m r q", m=nm, r=3)
        dn = dpool.tile([NP, 768], FP, name="dn", tag="dn")
        for r in (0, 1, 2):
            eng.dma_start(out=sw[:, :, r, :], in_=rsv[:, r, :, :])
        eng.dma_start(out=dn[:], in_=scr[:])
        dnv = dn[:].rearrange("p (r q) -> p r q", r=3)
        den = dpool.tile([NP, BS], FP, name="den", tag="den")
        num = dpool.tile([NP, BS], FP, name="num", tag="num")
        nc.vector.tensor_add(num[:], dnv[:, 1, :], dnv[:, 2, :])
        nc.vector.reciprocal_approx_fast(out=den[:], in_=dnv[:, 0, :])
        ov = dpool.tile([NP, BS], FP, name="ov", tag="ov")
        nc.vector.tensor_mul(ov[:], num[:], den[:])
        n0 = 4 * m0
        eng.dma_start(
            out=out_t[n0:n0 + NP, b0 * BS:(b0 + 1) * BS], in_=ov[:]
        )
        if half in (None, 1):
            res_state["nflush"] = 0
            res_state["rs"] = None

    pending = None
    cur_qs = None
    for G in range(NGRP):
        gpairs = [g for g in range(GROUP * G, min(GROUP * (G + 1), PAIRS))]
        if G % SPG == 0:
            cur_qs = load_stage(G // SPG)
        wi = G % SPG
        qsv = cur_qs[:]
        cur_stage = ps_stage.tile([128, GROUP * 512], FP, name="st", tag="st")
        members = []
        for s, g in enumerate(gpairs):
            if g % N == 0:
                vcur[0] = load_vtile(g // N)
            members.append((s, g // N, g % N, vcur[0]))
        # waves of 3 co-streaming row-group-tiled score matmuls (t-outer:
        # a pair's two halves share a row group and serialize; distinct
        # pairs use distinct row groups AND distinct PSUM banks)
        for t in (0, 1):
            for s, g in enumerate(gpairs):
                nc.tensor.matmul(
                    cur_stage[:, s * 512 + t * 256: s * 512 + (t + 1) * 256],
                    lhsT=qsv[32 * s:32 * s + 3,
                             wi * PW + 256 + t * 128: wi * PW + 256 + (t + 1) * 128],
                    rhs=qsv[32 * s:32 * s + 3, wi * PW: wi * PW + 256],
                    start=True, stop=True,
                    tile_position=(32 * s, 0),
                )
        e = epool.tile([128, GROUP * 512], BF16, name="e", tag="e")
        width = len(members) * 512
        nc.scalar.activation(e[:][:, 0:width], cur_stage[:][:, 0:width], AF.Exp)
        if pending is not None:
            emit_reduces(pending)
        pending = (e, members)
    emit_reduces(pending)
    assert res_state["count"] == 0 and res_state["nflush"] == 0, (
        "pair count must be a multiple of 32 (one block per division batch)"
    )


_CACHE: dict = {}


def _get_nc(reps: int = 1) -> bass.Bass:
    if reps not in _CACHE:
        _CACHE[reps] = build_kernel_module(reps)
    return _CACHE[reps]


def _split_bf16(x):
    hi = x.astype(BF_NP)
    lo = (x - hi.astype(np.float32)).astype(BF_NP)
    return hi, lo


def make_in_maps(query, key, value, in_proj_w, in_proj_b, out_proj_w, out_proj_b):
    q = np.asarray(query, dtype=np.float32).reshape(L, N)
    k = np.asarray(key, dtype=np.float32).reshape(L, N)
    vv = np.asarray(value, dtype=np.float32).reshape(L, N)
    wq, wk, wv = [float(x) for x in np.asarray(in_proj_w, dtype=np.float32).reshape(3)]
    bq, bk, bv = [float(x) for x in np.asarray(in_proj_b, dtype=np.float32).reshape(3)]
    wo = float(np.asarray(out_proj_w, dtype=np.float32).reshape(1)[0])
    bo = float(np.asarray(out_proj_b, dtype=np.float32).reshape(1)[0])

    qp = q * np.float32(wq) + np.float32(bq)
    kp = k * np.float32(wk) + np.float32(bk)
    # softmax weights sum to 1 -> the whole v/out affine folds into v:
    veff = (vv * np.float32(wv) + np.float32(bv)) * np.float32(wo) + np.float32(bo)

    qhi, qlo = _split_bf16(qp)
    khi, klo = _split_bf16(kp)
    vhi, vlo = _split_bf16(veff)

    in_maps = []
    for c in range(NCORES):
        sl = slice(c * LS, (c + 1) * LS)
        # [LS, N] core shards -> per-pair vectors; pair g = b*N + n
        def pairs_of(x):
            # -> [PAIRS, BS] (pair-major), x is [LS, N]
            return np.ascontiguousarray(
                x[sl].reshape(BPC, BS, N).transpose(0, 2, 1).reshape(PAIRS, BS)
            )

        qh, ql = pairs_of(qhi), pairs_of(qlo)
        kh, kl = pairs_of(khi), pairs_of(klo)
        # row 3s+c <-> SBUF partition 32s+c; col slot = act group G; pair 3G+s
        qkst = np.zeros((3, 3, NGRP, PW), dtype=BF_NP)
        for s in range(3):
            sel = slice(s, PAIRS, 3)          # pairs 3G+s
            cnt = len(range(PAIRS)[sel])
            qkst[s, 0, :cnt, 0:256] = qh[sel]
            qkst[s, 1, :cnt, 0:256] = qh[sel]
            qkst[s, 2, :cnt, 0:256] = ql[sel]
            qkst[s, 0, :cnt, 256:512] = kh[sel]
            qkst[s, 1, :cnt, 256:512] = kl[sel]
            qkst[s, 2, :cnt, 256:512] = kh[sel]
        qkst = np.ascontiguousarray(qkst.reshape(9, NGRP * PW))

        # vtab[p, (b, t, n, c)] with c = (1, vhi, vlo)
        vt = np.empty((128, BPC, 2, N, 3), dtype=BF_NP)
        vt[:, :, :, :, 0] = np.float32(1.0)
        # vhi[sl] is [LS, N] = [(b t p), n]
        vt[:, :, :, :, 1] = vhi[sl].reshape(BPC, 2, 128, N).transpose(2, 0, 1, 3)
        vt[:, :, :, :, 2] = vlo[sl].reshape(BPC, 2, 128, N).transpose(2, 0, 1, 3)
        vt = np.ascontiguousarray(vt.reshape(128, BPC * 2 * N * 3))

        in_maps.append({"qkst": qkst, "vtab": vt})
    return in_maps, None


def run(in_maps, sc=None, **kwargs):
    return run_bass_kernel_spmd(_get_nc(), in_maps, list(range(NCORES)), **kwargs)


def assemble(results) -> np.ndarray:
    outs = [np.asarray(results[c]["out_t"], dtype=np.float32).T for c in range(NCORES)]
    return np.ascontiguousarray(np.concatenate(outs, axis=0)).reshape(L, N, 1)


def kernel(query, key, value, in_proj_w, in_proj_b, out_proj_w, out_proj_b):
    in_maps, sc = make_in_maps(
        query, key, value, in_proj_w, in_proj_b, out_proj_w, out_proj_b
    )
    res = run(in_maps, sc)
    return assemble(res.results)
